# revision 1
# baseline (speedup 1.0000x reference)
"""GraphTransformer (TransformerConv + mean-pool) on 8 trn2 NeuronCores.

Strategy (two launches, nodes sharded 8 ways):
  Launch A (per core, 6250 nodes + pad -> 6272):
      h = x @ W_emb + b_emb           (computed transposed, hT, via W^T @ x^T)
      qkv = h @ [Wq|Wk|Wv] + b        -> bf16 [6272, 1536] per core
      skip = h @ Wskip + bskip        -> fp32 [6272, 64]  per core
  Host: assemble full Q,K,V; sort edges by dst; group per dst-tile (128 dst
      nodes, capacity 9*128 edge slots); gather per-edge rows
      qg=Q[dst], kg=K[src], vg=V[src]; build one-hot indicator matrices.
  Launch B (per core, 49 dst tiles x 9 chunks of 128 edges):
      s[e,h]   = sum_c qg[e,hc]*kg[e,hc]        (DVE mult + grouped reduce)
      w[e,h]   = exp(s*scale)                   (ACT, broadcast-expanded)
      num[d,:] += ind_ed^T @ (w*vg)             (TensorE scatter via one-hot)
      den[d,h] += ind_ed^T @ w
      out[d,:] = mean_h(num/den) + skip[d,:]
      pooled[g,:] += ind_ng^T @ out             (TensorE, per-graph partial)
  Host: sum partial pooled over cores, divide by graph node counts.
"""

import numpy as np
import ml_dtypes

import concourse.bass as bass
from concourse import bacc
import concourse.mybir as mybir
import concourse.tile as tile
from concourse import bass_utils
from concourse.bass import ts

BF16 = mybir.dt.bfloat16
F32 = mybir.dt.float32
NP_BF16 = ml_dtypes.bfloat16

N, E, B = 50000, 400000, 64
IN_DIM, OUT_DIM, HEADS = 768, 64, 8
HC = HEADS * OUT_DIM  # 512
NCORES = 8
NPC = N // NCORES  # 6250 nodes per core
TILES = 49  # dst tiles per core (49*128 = 6272 >= 6250)
NPAD = TILES * 128  # 6272
CHUNKS = 9  # edge chunks (of 128) per dst tile
CAP = CHUNKS * 128  # 1152 edge slots per tile
KCH = IN_DIM // 128  # 6 contraction chunks

TRACE = False
LAST_EXEC_NS = {}

_cache = {}


def _build_launch_a():
    # qkv/skip computed directly from x with host-fused weights:
    #   W_eff = W_emb @ [Wq|Wk|Wv|Wskip],  b_eff = b_emb @ [..] + [bq|bk|bv|bskip]
    nc = bacc.Bacc("TRN2", debug=False, num_devices=NCORES)
    xT = nc.dram_tensor("xT", [KCH * 128, NPAD], BF16, kind="ExternalInput").ap()
    wqkvs = nc.dram_tensor("wqkvs", [KCH * 128, 1600], BF16, kind="ExternalInput").ap()
    bqkvs = nc.dram_tensor("bqkvs", [128, 1600], BF16, kind="ExternalInput").ap()
    qkv_out = nc.dram_tensor("qkv_out", [NPAD, 1536], BF16, kind="ExternalOutput").ap()
    skip_out = nc.dram_tensor("skip_out", [NPAD, OUT_DIM], F32, kind="ExternalOutput").ap()

    with tile.TileContext(nc) as tc:
        with (
            tc.tile_pool(name="const", bufs=1) as cpool,
            tc.tile_pool(name="work", bufs=3) as wpool,
            tc.tile_pool(name="psum_qkv", bufs=2, space="PSUM") as pq,
        ):
            xT_sb = cpool.tile([128, KCH * NPAD], BF16)
            wqkvs_sb = cpool.tile([128, KCH * 1600], BF16)
            bqkvs_sb = cpool.tile([128, 1600], BF16)
            for k in range(KCH):
                nc.sync.dma_start(xT_sb[:, k * NPAD:(k + 1) * NPAD], xT[ts(k, 128), :])
                nc.sync.dma_start(wqkvs_sb[:, k * 1600:(k + 1) * 1600], wqkvs[ts(k, 128), :])
            nc.sync.dma_start(bqkvs_sb[:], bqkvs[:])

            for m in range(TILES):
                qkvs_ps = pq.tile([128, 1600], F32, tag="qkvs")
                for k in range(KCH):
                    for n0, nw in ((0, 512), (512, 512), (1024, 512), (1536, 64)):
                        nc.tensor.matmul(
                            qkvs_ps[:, n0:n0 + nw],
                            lhsT=xT_sb[:, k * NPAD + m * 128: k * NPAD + (m + 1) * 128],
                            rhs=wqkvs_sb[:, k * 1600 + n0: k * 1600 + n0 + nw],
                            start=(k == 0),
                            stop=(k == KCH - 1),
                        )
                qkv_sb = wpool.tile([128, 1536], BF16, tag="qkv")
                nc.vector.tensor_add(qkv_sb[:], qkvs_ps[:, :1536], bqkvs_sb[:, :1536])
                skip_sb = wpool.tile([128, OUT_DIM], F32, tag="skip")
                nc.vector.tensor_add(skip_sb[:], qkvs_ps[:, 1536:1600], bqkvs_sb[:, 1536:1600])
                nc.sync.dma_start(qkv_out[ts(m, 128), :], qkv_sb[:])
                nc.sync.dma_start(skip_out[ts(m, 128), :], skip_sb[:])
    nc.compile()
    return nc


def _build_launch_b():
    nc = bacc.Bacc("TRN2", debug=False, num_devices=NCORES)
    qg = nc.dram_tensor("qg", [TILES, 128, CHUNKS * HC], BF16, kind="ExternalInput").ap()
    kg = nc.dram_tensor("kg", [TILES, 128, CHUNKS * HC], BF16, kind="ExternalInput").ap()
    vg = nc.dram_tensor("vg", [TILES, 128, CHUNKS * HC], BF16, kind="ExternalInput").ap()
    ind = nc.dram_tensor("ind", [TILES, 128, CHUNKS * 128], BF16, kind="ExternalInput").ap()
    skip = nc.dram_tensor("skip", [TILES, 128, OUT_DIM], F32, kind="ExternalInput").ap()
    indng = nc.dram_tensor("indng", [TILES, 128, B], BF16, kind="ExternalInput").ap()
    pooled = nc.dram_tensor("pooled", [B, OUT_DIM], F32, kind="ExternalOutput").ap()

    scale = 1.0 / np.sqrt(np.float32(OUT_DIM))

    with tile.TileContext(nc) as tc:
        with (
            tc.tile_pool(name="io", bufs=3) as iop,
            tc.tile_pool(name="work", bufs=4) as wp,
            tc.tile_pool(name="psA", bufs=2, space="PSUM") as psA,
            tc.tile_pool(name="psB", bufs=2, space="PSUM") as psB,
            tc.tile_pool(name="psP", bufs=1, space="PSUM") as psP,
            tc.tile_pool(name="outp", bufs=1) as outp,
        ):
            pool_ps = psP.tile([B, OUT_DIM], F32)
            for t in range(TILES):
                qg_sb = iop.tile([128, CHUNKS * HC], BF16, tag="qg")
                kg_sb = iop.tile([128, CHUNKS * HC], BF16, tag="kg")
                vg_sb = iop.tile([128, CHUNKS * HC], BF16, tag="vg")
                ind_sb = iop.tile([128, CHUNKS * 128], BF16, tag="ind")
                skip_sb = iop.tile([128, OUT_DIM], F32, tag="skip")
                indng_sb = iop.tile([128, B], BF16, tag="indng")
                nc.sync.dma_start(qg_sb[:], qg[t])
                nc.sync.dma_start(kg_sb[:], kg[t])
                nc.sync.dma_start(vg_sb[:], vg[t])
                nc.sync.dma_start(ind_sb[:], ind[t])
                nc.sync.dma_start(skip_sb[:], skip[t])
                nc.sync.dma_start(indng_sb[:], indng[t])

                num_ps = psA.tile([128, HC], F32, tag="num")
                den_ps = psB.tile([128, HEADS], F32, tag="den")
                # process chunks in pairs: one DVE/ACT instruction covers 2 chunks
                for c0 in range(0, CHUNKS, 2):
                    w2 = min(2, CHUNKS - c0)
                    wd = w2 * HC
                    qk = wp.tile([128, 2 * HC], BF16, tag="qk")
                    nc.vector.tensor_mul(
                        qk[:, :wd], qg_sb[:, c0 * HC:(c0 + w2) * HC],
                        kg_sb[:, c0 * HC:(c0 + w2) * HC])
                    # fold halves at DVE 2x, then half-size reduce at 1x
                    qkh = wp.tile([128, HC], BF16, tag="qkh")
                    h3 = qk[:, :wd].rearrange("p (h c) -> p h c", h=w2 * HEADS)
                    nc.vector.tensor_add(
                        qkh[:, :wd // 2].rearrange("p (h c) -> p h c", h=w2 * HEADS),
                        h3[:, :, 0:OUT_DIM // 2], h3[:, :, OUT_DIM // 2:OUT_DIM])
                    s_f = wp.tile([128, 2 * HEADS], F32, tag="s")
                    nc.vector.reduce_sum(
                        s_f[:, :w2 * HEADS],
                        qkh[:, :wd // 2].rearrange("p (h c) -> p h c", h=w2 * HEADS),
                        axis=mybir.AxisListType.X,
                    )
                    w_bf = wp.tile([128, 2 * HC], BF16, tag="w")
                    nc.scalar.activation(
                        out=w_bf[:, :wd].rearrange("p (h c) -> p h c", h=w2 * HEADS),
                        in_=s_f[:, :w2 * HEADS].rearrange("p h -> p h ()").to_broadcast(
                            [128, w2 * HEADS, OUT_DIM]),
                        func=mybir.ActivationFunctionType.Exp,
                        scale=float(scale),
                    )
                    wv = wp.tile([128, 2 * HC], BF16, tag="wv")
                    nc.vector.tensor_mul(wv[:, :wd], vg_sb[:, c0 * HC:(c0 + w2) * HC], w_bf[:, :wd])
                    for c in range(c0, c0 + w2):
                        j = c - c0
                        nc.tensor.matmul(
                            num_ps[:], lhsT=ind_sb[:, ts(c, 128)], rhs=wv[:, ts(j, HC)],
                            start=(c == 0), stop=(c == CHUNKS - 1),
                        )
                        nc.tensor.matmul(
                            den_ps[:], lhsT=ind_sb[:, ts(c, 128)],
                            rhs=w_bf[:, ts(j, HC)].rearrange("p (h c) -> p h c", h=HEADS)[:, :, 0],
                            start=(c == 0), stop=(c == CHUNKS - 1),
                        )
                # epilogue: out = mean_h(num/den) + skip
                rec = wp.tile([128, HEADS], F32, tag="rec")
                nc.vector.tensor_scalar(
                    out=rec[:], in0=den_ps[:],
                    scalar1=float(HEADS), scalar2=1e-12,
                    op0=mybir.AluOpType.mult, op1=mybir.AluOpType.add,
                )
                nc.vector.reciprocal(rec[:], rec[:])
                mh = wp.tile([128, HC], BF16, tag="mh")
                nc.vector.tensor_mul(
                    mh[:].rearrange("p (h c) -> p h c", h=HEADS),
                    num_ps[:].rearrange("p (h c) -> p h c", h=HEADS),
                    rec[:].rearrange("p h -> p h ()").to_broadcast([128, HEADS, OUT_DIM]),
                )
                out_sb = wp.tile([128, OUT_DIM], BF16, tag="out")
                msum = wp.tile([128, OUT_DIM], F32, tag="msum")
                nc.vector.reduce_sum(
                    msum[:],
                    mh[:].rearrange("p (h c) -> p c h", h=HEADS),
                    axis=mybir.AxisListType.X,
                )
                nc.vector.tensor_add(out_sb[:], msum[:], skip_sb[:])
                nc.tensor.matmul(
                    pool_ps[:], lhsT=indng_sb[:], rhs=out_sb[:],
                    start=(t == 0), stop=(t == TILES - 1),
                )
            pooled_sb = outp.tile([B, OUT_DIM], F32)
            nc.vector.tensor_copy(pooled_sb[:], pool_ps[:])
            nc.sync.dma_start(pooled[:], pooled_sb[:])
    nc.compile()
    return nc


def _get_programs():
    if "A" not in _cache:
        _cache["A"] = _build_launch_a()
    if "B" not in _cache:
        _cache["B"] = _build_launch_b()
    return _cache["A"], _cache["B"]


LAST_TRACE_PATH = {}


def _ensure_hook_shim():
    import sys
    import types

    if "antenv.axon_hooks" in sys.modules:
        return
    mod = types.ModuleType("antenv.axon_hooks")
    holder = [None]
    mod.set_axon_ntff_profile_hook = lambda h: holder.__setitem__(0, h)
    mod.get_axon_ntff_profile_hook = lambda: holder[0]
    sys.modules["antenv.axon_hooks"] = mod
    import antenv

    antenv.axon_hooks = mod
    from trn_agent_boot.trn_boot import _ntff_profile_via_ctypes

    mod.set_axon_ntff_profile_hook(
        _ntff_profile_via_ctypes("/opt/axon/libaxon_pjrt.so")
    )


def _run(nc, in_maps, label):
    if not TRACE:
        res = bass_utils.run_bass_kernel_spmd(nc, in_maps, list(range(NCORES)))
        return res.results

    import glob
    import os
    import tempfile

    from concourse import bass2jax
    from concourse._compat import FishPath
    import gauge.profiler

    _ensure_hook_shim()
    import antenv.axon_hooks as hooks

    tmpdir = tempfile.mkdtemp(prefix=f"bass_{label}_")
    with hooks.get_axon_ntff_profile_hook()(tmpdir, [0]):
        results = bass2jax.run_bass_via_pjrt(nc, in_maps, n_cores=NCORES)
    exec_ns = None
    try:
        ntffs = glob.glob(os.path.join(tmpdir, "*_body*.ntff"))
        if ntffs:
            profile = gauge.profiler.Profile(
                profile_path=FishPath(tmpdir),
                kernel_dev_mode=True,
                profile_on_exit=False,
                bass_kernel=nc.m,
                offline_processing=True,
                fname="*_body*",
            )
            prs = profile.to_perfetto(model_index=(0,))
            if prs:
                exec_ns = max(p.exec_time_ns for p in prs)
                LAST_TRACE_PATH[label] = (tmpdir, [p.trace_path for p in prs])
        else:
            print(f"[{label}] no ntff files in {tmpdir}: {os.listdir(tmpdir)}")
    except Exception as e:  # profiling must never break the run
        print(f"[{label}] profile processing failed: {type(e).__name__}: {e}")
    LAST_EXEC_NS[label] = exec_ns
    return results


def kernel(x, edge_index, batch, W_emb, b_emb, Wq, bq, Wk, bk, Wv, bv, Wskip, bskip):
    x = np.asarray(x, np.float32)
    edge_index = np.asarray(edge_index)
    batch_np = np.asarray(batch)
    ncA, ncB = _get_programs()

    # ---- host prep for launch A: fold W_emb/b_emb into the qkv/skip weights ----
    wcat = np.concatenate(
        [np.asarray(Wq, np.float32), np.asarray(Wk, np.float32),
         np.asarray(Wv, np.float32), np.asarray(Wskip, np.float32)], axis=1
    )  # [768, 1600]
    bcat = np.concatenate(
        [np.asarray(bq, np.float32), np.asarray(bk, np.float32),
         np.asarray(bv, np.float32), np.asarray(bskip, np.float32)]
    )  # [1600]
    wemb_f = np.asarray(W_emb, np.float32)
    bemb_f = np.asarray(b_emb, np.float32)
    wqkvs = (wemb_f @ wcat).astype(NP_BF16)          # [768, 1600]
    bqkvs = (bemb_f @ wcat + bcat).astype(np.float32)
    bqkvs_rep = np.broadcast_to(bqkvs.astype(NP_BF16), (128, 1600)).copy()

    xpad = np.zeros((NCORES * NPAD, IN_DIM), NP_BF16)
    for c in range(NCORES):
        xpad[c * NPAD: c * NPAD + NPC] = x[c * NPC:(c + 1) * NPC].astype(NP_BF16)
    in_maps_a = []
    for c in range(NCORES):
        xT = np.ascontiguousarray(xpad[c * NPAD:(c + 1) * NPAD].T)  # [768, 6272]
        in_maps_a.append({"xT": xT, "wqkvs": wqkvs, "bqkvs": bqkvs_rep})
    res_a = _run(ncA, in_maps_a, "A")

    # ---- host mid: assemble Q,K,V and build edge-sorted gathers ----
    Q = np.concatenate([res_a[c]["qkv_out"][:NPC, 0:512] for c in range(NCORES)])
    K = np.concatenate([res_a[c]["qkv_out"][:NPC, 512:1024] for c in range(NCORES)])
    V = np.concatenate([res_a[c]["qkv_out"][:NPC, 1024:1536] for c in range(NCORES)])

    src = np.asarray(edge_index[0], np.int64)
    dst = np.asarray(edge_index[1], np.int64)
    core = dst // NPC
    local = dst - core * NPC
    tile_g = core * TILES + local // 128  # 0 .. 8*49-1
    dloc = local % 128
    order = np.argsort(tile_g, kind="stable")
    tg_s, src_s, dloc_s = tile_g[order], src[order], dloc[order]
    ntile = NCORES * TILES
    counts = np.bincount(tg_s, minlength=ntile)
    if counts.max() > CAP:
        raise RuntimeError(f"tile capacity exceeded: {counts.max()} > {CAP}")
    starts = np.zeros(ntile, np.int64)
    starts[1:] = np.cumsum(counts)[:-1]
    pos = np.arange(E) - starts[tg_s]
    rows = tg_s * CAP + pos  # slot in [ntile*CAP]

    src_pad = np.zeros(ntile * CAP, np.int64)
    src_pad[rows] = src_s
    dst_pad = np.full(ntile * CAP, -1, np.int64)
    dst_pad[rows] = dloc_s
    dstg_pad = np.zeros(ntile * CAP, np.int64)
    dstg_pad[rows] = dst[order]

    def tileize(a):  # [ntile*CAP, D] -> [ntile, 128, CHUNKS*D]
        d = a.shape[1]
        return np.ascontiguousarray(
            a.reshape(ntile, CHUNKS, 128, d).transpose(0, 2, 1, 3).reshape(ntile, 128, CHUNKS * d)
        )

    qg_t = tileize(Q[dstg_pad])
    kg_t = tileize(K[src_pad])
    vg_t = tileize(V[src_pad])
    ind_t = tileize(
        (dst_pad[:, None] == np.arange(128)[None, :]).astype(NP_BF16)
    )

    batch_pad = np.full(NCORES * NPAD, -1, np.int64)
    for c in range(NCORES):
        batch_pad[c * NPAD: c * NPAD + NPC] = batch_np[c * NPC:(c + 1) * NPC]
    indng = (batch_pad[:, None] == np.arange(B)[None, :]).astype(NP_BF16)
    indng = indng.reshape(NCORES, TILES, 128, B)

    in_maps_b = []
    for c in range(NCORES):
        sl = slice(c * TILES, (c + 1) * TILES)
        in_maps_b.append({
            "qg": qg_t[sl], "kg": kg_t[sl], "vg": vg_t[sl], "ind": ind_t[sl],
            "skip": np.ascontiguousarray(
                res_a[c]["skip_out"].reshape(TILES, 128, OUT_DIM)),
            "indng": indng[c],
        })
    res_b = _run(ncB, in_maps_b, "B")

    pooled = np.zeros((B, OUT_DIM), np.float64)
    for c in range(NCORES):
        pooled += res_b[c]["pooled"].astype(np.float64)
    cnt = np.bincount(batch_np, minlength=B).astype(np.float64)
    pooled /= np.maximum(cnt, 1.0)[:, None]
    return pooled.astype(np.float32)



# revision 4
# speedup vs baseline: 1.1838x; 1.1838x over previous
"""GraphTransformer (TransformerConv + mean-pool) on 8 trn2 NeuronCores.

Strategy (two launches):
  Launch A (per core, 6250 nodes + pad -> 6272):
      qkv/skip = x @ fused(W_emb,[Wq|Wk|Wv|Wskip]) + fused bias   (bf16 TensorE)
  Host: assemble Q,K,V (cast fp8), sort nodes by dst-degree, stripe across
      8 cores into 49 tiles x 128 lanes; chunk c of a tile holds the c-th
      incident edge of each lane's node (pad rows zero).  Per-tile chunk
      count C_t = max degree in the tile's degree window (SPMD-identical
      across cores).  Ship per-edge K rows transposed (kgT, channel on
      partitions) and V rows channel-major (vg), plus Q transposed per tile.
  Launch B (per core, 49 tiles, passes of <=PCAP chunks):
      qk   = kgT * qT(bcast)                  DVE 2x
      s    = headsel^T @ qk                   TensorE (head-sum over channels)
      w    = exp(s*scale)                     ACT, [8, P*128] only
      w2   = transpose(w) via identity matmul TensorE -> [128, C*8]
      wv   = vg * w2(bcast)                   DVE 2x (c-major keeps stride-1)
      num += I128 @ wv                        TensorE PSUM accumulate
      den  = sum_c w2 - npad;  out = mean_h(num/(8*den)) + skip
      pooled += indng^T @ out                 TensorE per-graph partial
  Host: sum partial pooled over cores, divide by graph node counts.
"""

import math

import numpy as np
import ml_dtypes

import concourse.bass as bass
from concourse import bacc
import concourse.mybir as mybir
import concourse.tile as tile
from concourse import bass_utils
from concourse.bass import ts

BF16 = mybir.dt.bfloat16
F32 = mybir.dt.float32
F8 = mybir.dt.float8e4
NP_BF16 = ml_dtypes.bfloat16
NP_F8 = ml_dtypes.float8_e4m3fn

N, E, B = 50000, 400000, 64
IN_DIM, OUT_DIM, HEADS = 768, 64, 8
HC = HEADS * OUT_DIM  # 512
NCORES = 8
NPC = N // NCORES  # 6250 nodes per core (launch A sharding)
TILES = 49
NPAD = TILES * 128  # 6272
NSLOT = NCORES * NPAD  # 50176
KCH = IN_DIM // 128  # 6 contraction chunks (launch A)
PCAP = 12  # max chunks per pass in launch B

TRACE = False
LAST_EXEC_NS = {}
LAST_TRACE_PATH = {}

_cache = {}


def _build_launch_a():
    # qkv/skip computed directly from x with host-fused weights:
    #   W_eff = W_emb @ [Wq|Wk|Wv|Wskip],  b_eff = b_emb @ [..] + [bq|bk|bv|bskip]
    nc = bacc.Bacc("TRN2", debug=False, num_devices=NCORES)
    xT = nc.dram_tensor("xT", [KCH * 128, NPAD], BF16, kind="ExternalInput").ap()
    wqkvs = nc.dram_tensor("wqkvs", [KCH * 128, 1600], BF16, kind="ExternalInput").ap()
    bqkvs = nc.dram_tensor("bqkvs", [128, 1600], BF16, kind="ExternalInput").ap()
    qkv_out = nc.dram_tensor("qkv_out", [NPAD, 1536], BF16, kind="ExternalOutput").ap()
    skip_out = nc.dram_tensor("skip_out", [NPAD, OUT_DIM], F32, kind="ExternalOutput").ap()

    with tile.TileContext(nc) as tc:
        with (
            tc.tile_pool(name="const", bufs=1) as cpool,
            tc.tile_pool(name="work", bufs=3) as wpool,
            tc.tile_pool(name="psum_qkv", bufs=2, space="PSUM") as pq,
        ):
            xT_sb = cpool.tile([128, KCH * NPAD], BF16)
            wqkvs_sb = cpool.tile([128, KCH * 1600], BF16)
            bqkvs_sb = cpool.tile([128, 1600], BF16)
            for k in range(KCH):
                nc.sync.dma_start(xT_sb[:, k * NPAD:(k + 1) * NPAD], xT[ts(k, 128), :])
                nc.sync.dma_start(wqkvs_sb[:, k * 1600:(k + 1) * 1600], wqkvs[ts(k, 128), :])
            nc.sync.dma_start(bqkvs_sb[:], bqkvs[:])

            for m in range(TILES):
                qkvs_ps = pq.tile([128, 1600], F32, tag="qkvs")
                for k in range(KCH):
                    for n0, nw in ((0, 512), (512, 512), (1024, 512), (1536, 64)):
                        nc.tensor.matmul(
                            qkvs_ps[:, n0:n0 + nw],
                            lhsT=xT_sb[:, k * NPAD + m * 128: k * NPAD + (m + 1) * 128],
                            rhs=wqkvs_sb[:, k * 1600 + n0: k * 1600 + n0 + nw],
                            start=(k == 0),
                            stop=(k == KCH - 1),
                        )
                qkv_sb = wpool.tile([128, 1536], BF16, tag="qkv")
                nc.vector.tensor_add(qkv_sb[:], qkvs_ps[:, :1536], bqkvs_sb[:, :1536])
                skip_sb = wpool.tile([128, OUT_DIM], F32, tag="skip")
                nc.vector.tensor_add(skip_sb[:], qkvs_ps[:, 1536:1600], bqkvs_sb[:, 1536:1600])
                nc.sync.dma_start(qkv_out[ts(m, 128), :], qkv_sb[:])
                nc.sync.dma_start(skip_out[ts(m, 128), :], skip_sb[:])
    nc.compile()
    return nc


def _passes(c):
    return [(p0, min(PCAP, c - p0)) for p0 in range(0, c, PCAP)]


def _build_launch_b(c_list):
    nc = bacc.Bacc("TRN2", debug=False, num_devices=NCORES)
    cmax = max(max(c_list), 1)
    free_tot = sum(512 + c * 1024 for c in c_list)
    kvq = nc.dram_tensor("kvq", [128, free_tot], F8, kind="ExternalInput").ap()
    sknp = nc.dram_tensor("sknp", [TILES, 128, 66], F32, kind="ExternalInput").ap()
    indng = nc.dram_tensor("indng", [TILES, 128, B], BF16, kind="ExternalInput").ap()
    hsel = nc.dram_tensor("hsel", [128, 8], BF16, kind="ExternalInput").ap()
    identt = nc.dram_tensor("identt", [128, 128], BF16, kind="ExternalInput").ap()
    i8t = nc.dram_tensor("i8t", [8, 8], BF16, kind="ExternalInput").ap()
    pooled = nc.dram_tensor("pooled", [B, OUT_DIM], F32, kind="ExternalOutput").ap()

    scale = 1.0 / math.sqrt(float(OUT_DIM))

    with tile.TileContext(nc) as tc:
        with (
            tc.tile_pool(name="const", bufs=1) as cp,
            tc.tile_pool(name="io", bufs=2) as iop,
            tc.tile_pool(name="tio", bufs=2) as tiop,
            tc.tile_pool(name="wk", bufs=3) as wk,
            tc.tile_pool(name="ps_s", bufs=2, space="PSUM") as ps_s,
            tc.tile_pool(name="ps_w2", bufs=2, space="PSUM") as ps_w2,
            tc.tile_pool(name="ps_num", bufs=2, space="PSUM") as ps_num,
            tc.tile_pool(name="ps_pool", bufs=1, space="PSUM") as ps_pool,
            tc.tile_pool(name="outp", bufs=1) as outp,
        ):
            hsel_sb = cp.tile([128, 8], BF16)
            nc.sync.dma_start(hsel_sb[:], hsel[:])
            ident_sb = cp.tile([128, 128], BF16)
            nc.sync.dma_start(ident_sb[:], identt[:])
            i8_sb = cp.tile([8, 8], BF16)
            nc.sync.dma_start(i8_sb[:], i8t[:])

            pool_ps = ps_pool.tile([B, OUT_DIM], F32)
            off = 0
            for t, C in enumerate(c_list):
                qt_sb = tiop.tile([128, 512], BF16, tag="qt")
                nc.gpsimd.dma_start(qt_sb[:], kvq[:, off:off + 512])
                off += 512
                sknp_sb = tiop.tile([128, 66], F32, tag="sknp")
                nc.sync.dma_start(sknp_sb[:], sknp[t])
                indng_sb = tiop.tile([128, B], BF16, tag="ing")
                nc.sync.dma_start(indng_sb[:], indng[t])

                w2_sb = wk.tile([128, cmax * 8], BF16, tag="w2")
                num_ps = ps_num.tile([128, HC], F32, tag="num")
                w2_ps = ps_w2.tile([128, cmax * 8], F32, tag="w2ps")
                qt4 = qt_sb.rearrange("p (b d) -> p b d", b=4)

                for (c0, P) in _passes(C):
                    kv_sb = iop.tile([128, PCAP * 1024], BF16, tag="kv")
                    nc.gpsimd.dma_start(kv_sb[:, :P * 1024], kvq[:, off:off + P * 1024])
                    off += P * 1024
                    kg4 = kv_sb[:, :P * 512].rearrange("p (b c d) -> p b c d", b=4, c=P)
                    vg4 = kv_sb[:, P * 512:P * 1024].rearrange(
                        "p (c f h) -> p c f h", c=P, f=OUT_DIM)

                    qk_sb = wk.tile([128, PCAP * 512], BF16, tag="qk")
                    qk4 = qk_sb[:, :P * 512].rearrange("p (b c d) -> p b c d", b=4, c=P)
                    for bb in range(4):
                        nc.vector.tensor_mul(
                            qk4[:, bb], kg4[:, bb],
                            qt4[:, bb].rearrange("p d -> p () d").to_broadcast([128, P, 128]),
                        )
                    wT_sb = wk.tile([8, PCAP * 128], BF16, tag="wT")
                    for p0 in range(0, P, 4):
                        pw = min(4, P - p0)
                        s_ps = ps_s.tile([8, 512], F32, tag="s")
                        for bb in range(4):
                            nc.tensor.matmul(
                                s_ps[:, :pw * 128],
                                lhsT=hsel_sb[:],
                                rhs=qk4[:, bb, p0:p0 + pw],
                                start=(bb == 0),
                                stop=(bb == 3),
                            )
                        nc.scalar.activation(
                            out=wT_sb[:, p0 * 128:(p0 + pw) * 128],
                            in_=s_ps[:, :pw * 128],
                            func=mybir.ActivationFunctionType.Exp,
                            scale=float(scale),
                        )
                    for c in range(P):
                        nc.tensor.matmul(
                            w2_ps[:, (c0 + c) * 8:(c0 + c + 1) * 8],
                            lhsT=wT_sb[:, c * 128:(c + 1) * 128],
                            rhs=i8_sb[:],
                            start=True,
                            stop=True,
                        )
                    nc.scalar.copy(
                        w2_sb[:, c0 * 8:(c0 + P) * 8], w2_ps[:, c0 * 8:(c0 + P) * 8])
                    wv_sb = wk.tile([128, PCAP * 512], BF16, tag="wv")
                    nc.vector.tensor_mul(
                        wv_sb[:, :P * 512].rearrange("p (c f h) -> p c f h", c=P, f=OUT_DIM),
                        vg4,
                        w2_sb[:, c0 * 8:(c0 + P) * 8].rearrange(
                            "p (c h) -> p c () h", c=P).to_broadcast([128, P, OUT_DIM, HEADS]),
                    )
                    for c in range(P):
                        nc.tensor.matmul(
                            num_ps[:],
                            lhsT=ident_sb[:],
                            rhs=wv_sb[:, c * 512:(c + 1) * 512],
                            start=(c0 + c == 0),
                            stop=(c0 + c == C - 1),
                        )

                out_sb = wk.tile([128, OUT_DIM], BF16, tag="out")
                if C > 0:
                    den_sb = wk.tile([128, 8], F32, tag="den")
                    nc.vector.reduce_sum(
                        den_sb[:],
                        w2_sb[:, :C * 8].rearrange("p (c h) -> p h c", c=C),
                        axis=mybir.AxisListType.X,
                    )
                    rec_sb = wk.tile([128, 8], BF16, tag="rec")
                    recf = wk.tile([128, 8], F32, tag="recf")
                    nc.vector.tensor_sub(
                        recf[:], den_sb[:],
                        sknp_sb[:, 64:65].to_broadcast([128, 8]),
                    )
                    nc.vector.tensor_scalar(
                        out=recf[:], in0=recf[:],
                        scalar1=float(HEADS), scalar2=1e-12,
                        op0=mybir.AluOpType.mult, op1=mybir.AluOpType.add,
                    )
                    with nc.allow_low_precision(reason="bf16 1/den keeps the mh multiply in DVE 2x mode"):
                        nc.vector.reciprocal(rec_sb[:], recf[:])
                    num_sb = wk.tile([128, HC], BF16, tag="numsb")
                    nc.scalar.copy(num_sb[:], num_ps[:])
                    mh_sb = wk.tile([128, HC], BF16, tag="mh")
                    nc.vector.tensor_mul(
                        mh_sb.rearrange("p (f h) -> p f h", f=OUT_DIM),
                        num_sb.rearrange("p (f h) -> p f h", f=OUT_DIM),
                        rec_sb.rearrange("p h -> p () h").to_broadcast([128, OUT_DIM, HEADS]),
                    )
                    msum_sb = wk.tile([128, OUT_DIM], F32, tag="msum")
                    nc.vector.reduce_sum(
                        msum_sb[:],
                        mh_sb.rearrange("p (f h) -> p f h", f=OUT_DIM),
                        axis=mybir.AxisListType.X,
                    )
                    nc.vector.tensor_add(out_sb[:], msum_sb[:], sknp_sb[:, :64])
                else:
                    nc.vector.tensor_copy(out_sb[:], sknp_sb[:, :64])
                nc.tensor.matmul(
                    pool_ps[:], lhsT=indng_sb[:], rhs=out_sb[:],
                    start=(t == 0), stop=(t == TILES - 1),
                )
            pooled_sb = outp.tile([B, OUT_DIM], F32)
            nc.vector.tensor_copy(pooled_sb[:], pool_ps[:])
            nc.sync.dma_start(pooled[:], pooled_sb[:])
    nc.compile()
    return nc


def _get_program_a():
    if "A" not in _cache:
        _cache["A"] = _build_launch_a()
    return _cache["A"]


def _get_program_b(c_list):
    key = ("B", tuple(c_list))
    if key not in _cache:
        _cache[key] = _build_launch_b(c_list)
    return _cache[key]


def _ensure_hook_shim():
    import sys
    import types

    if "antenv.axon_hooks" in sys.modules:
        return
    mod = types.ModuleType("antenv.axon_hooks")
    holder = [None]
    mod.set_axon_ntff_profile_hook = lambda h: holder.__setitem__(0, h)
    mod.get_axon_ntff_profile_hook = lambda: holder[0]
    sys.modules["antenv.axon_hooks"] = mod
    import antenv

    antenv.axon_hooks = mod
    from trn_agent_boot.trn_boot import _ntff_profile_via_ctypes

    mod.set_axon_ntff_profile_hook(
        _ntff_profile_via_ctypes("/opt/axon/libaxon_pjrt.so")
    )


def _run(nc, in_maps, label):
    if not TRACE:
        res = bass_utils.run_bass_kernel_spmd(nc, in_maps, list(range(NCORES)))
        return res.results

    import glob
    import os
    import tempfile

    from concourse import bass2jax
    from concourse._compat import FishPath
    import gauge.profiler

    _ensure_hook_shim()
    import antenv.axon_hooks as hooks

    tmpdir = tempfile.mkdtemp(prefix=f"bass_{label}_")
    with hooks.get_axon_ntff_profile_hook()(tmpdir, [0]):
        results = bass2jax.run_bass_via_pjrt(nc, in_maps, n_cores=NCORES)
    exec_ns = None
    try:
        ntffs = glob.glob(os.path.join(tmpdir, "*_body*.ntff"))
        if ntffs:
            profile = gauge.profiler.Profile(
                profile_path=FishPath(tmpdir),
                kernel_dev_mode=True,
                profile_on_exit=False,
                bass_kernel=nc.m,
                offline_processing=True,
                fname="*_body*",
            )
            prs = profile.to_perfetto(model_index=(0,))
            if prs:
                exec_ns = max(p.exec_time_ns for p in prs)
                LAST_TRACE_PATH[label] = (tmpdir, [p.trace_path for p in prs])
        else:
            print(f"[{label}] no ntff files in {tmpdir}: {os.listdir(tmpdir)}")
    except Exception as e:  # profiling must never break the run
        print(f"[{label}] profile processing failed: {type(e).__name__}: {e}")
    LAST_EXEC_NS[label] = exec_ns
    return results


def kernel(x, edge_index, batch, W_emb, b_emb, Wq, bq, Wk, bk, Wv, bv, Wskip, bskip):
    x = np.asarray(x, np.float32)
    edge_index = np.asarray(edge_index)
    batch_np = np.asarray(batch, np.int64)
    ncA = _get_program_a()

    # ---- host prep for launch A: fold W_emb/b_emb into the qkv/skip weights ----
    wcat = np.concatenate(
        [np.asarray(Wq, np.float32), np.asarray(Wk, np.float32),
         np.asarray(Wv, np.float32), np.asarray(Wskip, np.float32)], axis=1
    )  # [768, 1600]
    bcat = np.concatenate(
        [np.asarray(bq, np.float32), np.asarray(bk, np.float32),
         np.asarray(bv, np.float32), np.asarray(bskip, np.float32)]
    )  # [1600]
    wemb_f = np.asarray(W_emb, np.float32)
    bemb_f = np.asarray(b_emb, np.float32)
    wqkvs = (wemb_f @ wcat).astype(NP_BF16)          # [768, 1600]
    bqkvs = (bemb_f @ wcat + bcat).astype(np.float32)
    bqkvs_rep = np.broadcast_to(bqkvs.astype(NP_BF16), (128, 1600)).copy()

    xpad = np.zeros((NCORES * NPAD, IN_DIM), NP_BF16)
    for c in range(NCORES):
        xpad[c * NPAD: c * NPAD + NPC] = x[c * NPC:(c + 1) * NPC].astype(NP_BF16)
    in_maps_a = []
    for c in range(NCORES):
        xT = np.ascontiguousarray(xpad[c * NPAD:(c + 1) * NPAD].T)  # [768, 6272]
        in_maps_a.append({"xT": xT, "wqkvs": wqkvs, "bqkvs": bqkvs_rep})
    res_a = _run(ncA, in_maps_a, "A")

    # ---- host mid: assemble Q,K,V (fp8) + skip; build degree-grouped tiles ----
    qkv = np.concatenate([res_a[c]["qkv_out"][:NPC] for c in range(NCORES)])  # [N,1536] bf16
    SK = np.concatenate([res_a[c]["skip_out"][:NPC] for c in range(NCORES)])  # [N,64] f32

    Q8 = np.zeros((N + 1, HC), NP_F8)
    K8 = np.zeros((N + 1, HC), NP_F8)
    V8 = np.zeros((N + 1, HC), NP_F8)
    Q8[:N] = qkv[:, 0:512].astype(NP_F8)
    K8[:N] = qkv[:, 512:1024].astype(NP_F8)
    V8[:N] = qkv[:, 1024:1536].astype(NP_F8)

    src = np.asarray(edge_index[0], np.int64)
    dst = np.asarray(edge_index[1], np.int64)
    deg = np.bincount(dst, minlength=N)

    order = np.argsort(deg, kind="stable")  # ascending degree
    slot_node = np.full(NSLOT, N, np.int64)
    slot_node[176:] = order
    pos_of_node = np.empty(N, np.int64)
    pos_of_node[order] = 176 + np.arange(N)

    degslot = np.zeros(NSLOT, np.int64)
    degslot[176:] = deg[order]
    c_list = degslot.reshape(TILES, NCORES * 128).max(axis=1).tolist()
    c_list = [int(c) for c in c_list]
    choff = np.zeros(TILES + 1, np.int64)
    choff[1:] = np.cumsum(c_list)
    nch = int(choff[-1])

    # edge -> (core, tile, lane, chunk)
    pd = pos_of_node[dst]
    ecore = pd % NCORES
    er = pd // NCORES
    etile = er // 128
    elane = er % 128
    o = np.argsort(pd, kind="stable")
    pds = pd[o]
    uniq, grp_start = np.unique(pds, return_index=True)
    starts_per_edge = np.zeros(E, np.int64)
    starts_per_edge[grp_start] = grp_start
    starts_per_edge = np.maximum.accumulate(starts_per_edge)
    k_in_grp = np.arange(E) - starts_per_edge
    kchunk = np.empty(E, np.int64)
    kchunk[o] = k_in_grp

    src_grid = np.full((NCORES, nch, 128), N, np.int32)
    src_grid[ecore, choff[etile] + kchunk, elane] = src

    # node grid per (core, tile, lane)
    posg = (np.arange(TILES)[None, :, None] * (NCORES * 128)
            + np.arange(128)[None, None, :] * NCORES
            + np.arange(NCORES)[:, None, None])  # [core, tile, lane]
    node_grid = slot_node[posg]  # [NCORES, TILES, 128]

    SKz = np.zeros((N + 1, OUT_DIM), np.float32)
    SKz[:N] = SK
    degz = np.zeros(N + 1, np.int64)
    degz[:N] = deg
    batchz = np.full(N + 1, -1, np.int64)
    batchz[:N] = batch_np

    sknp_all = np.zeros((NCORES, TILES, 128, 66), np.float32)
    sknp_all[:, :, :, :64] = SKz[node_grid]
    sknp_all[:, :, :, 64] = (np.asarray(c_list)[None, :, None] - degz[node_grid])
    indng_all = (batchz[node_grid][:, :, :, None] == np.arange(B)[None, None, None, :]
                 ).astype(NP_BF16)

    hsel_np = (np.arange(128)[:, None] % 8 == np.arange(8)[None, :]).astype(NP_BF16)
    ident_np = np.eye(128, dtype=NP_BF16)
    i8_np = np.eye(8, dtype=NP_BF16)

    free_tot = sum(512 + c * 1024 for c in c_list)
    in_maps_b = []
    for c in range(NCORES):
        KG = K8[src_grid[c]]  # [nch, 128, 512] fp8
        VG = V8[src_grid[c]]
        kvq = np.empty((128, free_tot), NP_F8)
        fo = 0
        for t, C in enumerate(c_list):
            nodes_t = node_grid[c, t]  # [128]
            qrows = Q8[nodes_t]  # [128, 512]
            # qT: [d, h, b, l] -> [l, h, b, d] -> [128, 4*128]
            qT = qrows.reshape(128, 8, 4, 16).transpose(3, 1, 2, 0).reshape(128, 512)
            kvq[:, fo:fo + 512] = qT
            fo += 512
            for (p0, P) in _passes(C):
                kgp = KG[choff[t] + p0: choff[t] + p0 + P]  # [P, 128, 512]
                # kgT: [c, d, h, b, l] -> [l, h, b, c, d]
                kgT = kgp.reshape(P, 128, 8, 4, 16).transpose(4, 2, 3, 0, 1).reshape(
                    128, P * 512)
                kvq[:, fo:fo + P * 512] = kgT
                fo += P * 512
                vgp = VG[choff[t] + p0: choff[t] + p0 + P]  # [P, 128, 512]
                # vg c-major: [c, d, h, f] -> [d, c, f, h]
                vgc = vgp.reshape(P, 128, 8, 64).transpose(1, 0, 3, 2).reshape(
                    128, P * 512)
                kvq[:, fo:fo + P * 512] = vgc
                fo += P * 512
        assert fo == free_tot
        in_maps_b.append({
            "kvq": kvq,
            "sknp": sknp_all[c],
            "indng": indng_all[c],
            "hsel": hsel_np,
            "identt": ident_np,
            "i8t": i8_np,
        })

    ncB = _get_program_b(c_list)
    res_b = _run(ncB, in_maps_b, "B")

    pooled = np.zeros((B, OUT_DIM), np.float64)
    for c in range(NCORES):
        pooled += res_b[c]["pooled"].astype(np.float64)
    cnt = np.bincount(batch_np, minlength=B).astype(np.float64)
    pooled /= np.maximum(cnt, 1.0)[:, None]
    return pooled.astype(np.float32)


# revision 6
# speedup vs baseline: 1.4399x; 1.2164x over previous
"""GraphTransformer (TransformerConv + mean-pool) on 8 trn2 NeuronCores.

Strategy (two launches):
  Launch A (per core, 6250 nodes + pad -> 6272):
      qkv/skip = x @ fused(W_emb,[Wq|Wk|Wv|Wskip]) + fused bias   (bf16 TensorE)
  Host: assemble Q,K,V (cast fp8), sort nodes by dst-degree, stripe across
      8 cores into 49 tiles x 128 lanes; chunk c of a tile holds the c-th
      incident edge of each lane's node (pad rows zero).  Per-tile chunk
      count C_t = max degree in the tile's degree window (SPMD-identical
      across cores).  Ship per-edge K rows transposed (kgT, channel on
      partitions) and V rows channel-major (vg), plus Q transposed per tile.
  Launch B (per core, 49 tiles, passes of <=PCAP chunks):
      qk   = kgT * qT(bcast)                  DVE 2x
      s    = headsel^T @ qk                   TensorE (head-sum over channels)
      w    = exp(s*scale)                     ACT, [8, P*128] only
      w2   = transpose(w) via identity matmul TensorE -> [128, C*8]
      wv   = vg * w2(bcast)                   DVE 2x (c-major keeps stride-1)
      num += I128 @ wv                        TensorE PSUM accumulate
      den  = sum_c w2 - npad;  out = mean_h(num/(8*den)) + skip
      pooled += indng^T @ out                 TensorE per-graph partial
  Host: sum partial pooled over cores, divide by graph node counts.
"""

import math

import numpy as np
import ml_dtypes

import concourse.bass as bass
from concourse import bacc
import concourse.mybir as mybir
import concourse.tile as tile
from concourse import bass_utils
from concourse.bass import ts

BF16 = mybir.dt.bfloat16
F32 = mybir.dt.float32
F8 = mybir.dt.float8e4
NP_BF16 = ml_dtypes.bfloat16
NP_F8 = ml_dtypes.float8_e4m3fn

N, E, B = 50000, 400000, 64
IN_DIM, OUT_DIM, HEADS = 768, 64, 8
HC = HEADS * OUT_DIM  # 512
NCORES = 8
NPC = N // NCORES  # 6250 nodes per core (launch A sharding)
TILES = 49
NPAD = TILES * 128  # 6272
NSLOT = NCORES * NPAD  # 50176
KCH = IN_DIM // 128  # 6 contraction chunks (launch A)
PCAP = 12  # max chunks per pass in launch B

TRACE = False
LAST_EXEC_NS = {}
LAST_TRACE_PATH = {}

_cache = {}


def _build_launch_a():
    # qkv/skip computed directly from x with host-fused weights:
    #   W_eff = W_emb @ [Wq|Wk|Wv|Wskip],  b_eff = b_emb @ [..] + [bq|bk|bv|bskip]
    nc = bacc.Bacc("TRN2", debug=False, num_devices=NCORES)
    xT = nc.dram_tensor("xT", [KCH * 128, NPAD], BF16, kind="ExternalInput").ap()
    wqkvs = nc.dram_tensor("wqkvs", [KCH * 128, 1600], BF16, kind="ExternalInput").ap()
    bqkvs = nc.dram_tensor("bqkvs", [128, 1600], BF16, kind="ExternalInput").ap()
    qkv_out = nc.dram_tensor("qkv_out", [NPAD, 1536], BF16, kind="ExternalOutput").ap()
    skip_out = nc.dram_tensor("skip_out", [NPAD, OUT_DIM], F32, kind="ExternalOutput").ap()

    with tile.TileContext(nc) as tc:
        with (
            tc.tile_pool(name="const", bufs=1) as cpool,
            tc.tile_pool(name="work", bufs=3) as wpool,
            tc.tile_pool(name="psum_qkv", bufs=2, space="PSUM") as pq,
        ):
            xT_sb = cpool.tile([128, KCH * NPAD], BF16)
            wqkvs_sb = cpool.tile([128, KCH * 1600], BF16)
            bqkvs_sb = cpool.tile([128, 1600], BF16)
            for k in range(KCH):
                nc.sync.dma_start(xT_sb[:, k * NPAD:(k + 1) * NPAD], xT[ts(k, 128), :])
                nc.sync.dma_start(wqkvs_sb[:, k * 1600:(k + 1) * 1600], wqkvs[ts(k, 128), :])
            nc.sync.dma_start(bqkvs_sb[:], bqkvs[:])

            for m in range(TILES):
                qkvs_ps = pq.tile([128, 1600], F32, tag="qkvs")
                for k in range(KCH):
                    for n0, nw in ((0, 512), (512, 512), (1024, 512), (1536, 64)):
                        nc.tensor.matmul(
                            qkvs_ps[:, n0:n0 + nw],
                            lhsT=xT_sb[:, k * NPAD + m * 128: k * NPAD + (m + 1) * 128],
                            rhs=wqkvs_sb[:, k * 1600 + n0: k * 1600 + n0 + nw],
                            start=(k == 0),
                            stop=(k == KCH - 1),
                        )
                qkv_sb = wpool.tile([128, 1536], BF16, tag="qkv")
                nc.vector.tensor_add(qkv_sb[:], qkvs_ps[:, :1536], bqkvs_sb[:, :1536])
                skip_sb = wpool.tile([128, OUT_DIM], F32, tag="skip")
                nc.vector.tensor_add(skip_sb[:], qkvs_ps[:, 1536:1600], bqkvs_sb[:, 1536:1600])
                nc.sync.dma_start(qkv_out[ts(m, 128), :], qkv_sb[:])
                nc.sync.dma_start(skip_out[ts(m, 128), :], skip_sb[:])
    nc.compile()
    return nc


def _passes(c):
    return [(p0, min(PCAP, c - p0)) for p0 in range(0, c, PCAP)]


def _build_launch_b(c_list):
    nc = bacc.Bacc("TRN2", debug=False, num_devices=NCORES)
    cmax = max(max(c_list), 1)
    free_tot = sum(512 + c * 1024 for c in c_list)
    kvq = nc.dram_tensor("kvq", [128, free_tot], F8, kind="ExternalInput").ap()
    sknp = nc.dram_tensor("sknp", [TILES, 128, 66], F32, kind="ExternalInput").ap()
    indng = nc.dram_tensor("indng", [TILES, 128, B], BF16, kind="ExternalInput").ap()
    hsel = nc.dram_tensor("hsel", [128, 8], BF16, kind="ExternalInput").ap()
    identt = nc.dram_tensor("identt", [128, 128], BF16, kind="ExternalInput").ap()
    i8t = nc.dram_tensor("i8t", [8, 8], BF16, kind="ExternalInput").ap()
    pooled = nc.dram_tensor("pooled", [B, OUT_DIM], F32, kind="ExternalOutput").ap()

    scale = 1.0 / math.sqrt(float(OUT_DIM))

    with tile.TileContext(nc) as tc:
        with (
            tc.tile_pool(name="const", bufs=1) as cp,
            tc.tile_pool(name="io", bufs=3) as iop,
            tc.tile_pool(name="tio", bufs=3) as tiop,
            tc.tile_pool(name="wk", bufs=3) as wk,
            tc.tile_pool(name="ps_s", bufs=3, space="PSUM") as ps_s,
            tc.tile_pool(name="ps_w2", bufs=2, space="PSUM") as ps_w2,
            tc.tile_pool(name="ps_num", bufs=2, space="PSUM") as ps_num,
            tc.tile_pool(name="ps_pool", bufs=1, space="PSUM") as ps_pool,
            tc.tile_pool(name="outp", bufs=1) as outp,
        ):
            hsel_sb = cp.tile([128, 8], BF16)
            nc.sync.dma_start(hsel_sb[:], hsel[:])
            ident_sb = cp.tile([128, 128], BF16)
            nc.sync.dma_start(ident_sb[:], identt[:])
            i8_sb = cp.tile([8, 8], BF16)
            nc.sync.dma_start(i8_sb[:], i8t[:])

            pool_ps = ps_pool.tile([B, OUT_DIM], F32)
            off = 0
            for t, C in enumerate(c_list):
                qt_sb = tiop.tile([128, 512], BF16, tag="qt")
                nc.gpsimd.dma_start(qt_sb[:], kvq[:, off:off + 512])
                off += 512
                sknp_sb = tiop.tile([128, 66], F32, tag="sknp")
                nc.sync.dma_start(sknp_sb[:], sknp[t])
                indng_sb = tiop.tile([128, B], BF16, tag="ing")
                nc.sync.dma_start(indng_sb[:], indng[t])

                w2_sb = wk.tile([128, cmax * 8], BF16, tag="w2")
                num_ps = ps_num.tile([128, HC], F32, tag="num")
                w2_ps = ps_w2.tile([128, cmax * 8], F32, tag="w2ps")
                qt4 = qt_sb.rearrange("p (b d) -> p b d", b=4)

                for (c0, P) in _passes(C):
                    kv_sb = iop.tile([128, PCAP * 1024], BF16, tag="kv")
                    nc.gpsimd.dma_start(kv_sb[:, :P * 1024], kvq[:, off:off + P * 1024])
                    off += P * 1024
                    kg4 = kv_sb[:, :P * 512].rearrange("p (b c d) -> p b c d", b=4, c=P)
                    vg4 = kv_sb[:, P * 512:P * 1024].rearrange(
                        "p (c f h) -> p c f h", c=P, f=OUT_DIM)

                    qk_sb = wk.tile([128, PCAP * 512], BF16, tag="qk")
                    qk4 = qk_sb[:, :P * 512].rearrange("p (b c d) -> p b c d", b=4, c=P)
                    nc.vector.tensor_mul(
                        qk4, kg4,
                        qt4.rearrange("p b d -> p b () d").to_broadcast([128, 4, P, 128]),
                    )
                    wT_sb = wk.tile([8, PCAP * 128], BF16, tag="wT")
                    for p0 in range(0, P, 4):
                        pw = min(4, P - p0)
                        s_ps = ps_s.tile([8, 512], F32, tag="s")
                        for bb in range(4):
                            nc.tensor.matmul(
                                s_ps[:, :pw * 128],
                                lhsT=hsel_sb[:],
                                rhs=qk4[:, bb, p0:p0 + pw],
                                start=(bb == 0),
                                stop=(bb == 3),
                            )
                        nc.scalar.activation(
                            out=wT_sb[:, p0 * 128:(p0 + pw) * 128],
                            in_=s_ps[:, :pw * 128],
                            func=mybir.ActivationFunctionType.Exp,
                            scale=float(scale),
                        )
                    for c in range(P):
                        nc.tensor.matmul(
                            w2_ps[:, (c0 + c) * 8:(c0 + c + 1) * 8],
                            lhsT=wT_sb[:, c * 128:(c + 1) * 128],
                            rhs=i8_sb[:],
                            start=True,
                            stop=True,
                        )
                    nc.scalar.copy(
                        w2_sb[:, c0 * 8:(c0 + P) * 8], w2_ps[:, c0 * 8:(c0 + P) * 8])
                    wv_sb = wk.tile([128, PCAP * 512], BF16, tag="wv")
                    nc.vector.tensor_mul(
                        wv_sb[:, :P * 512].rearrange("p (c f h) -> p c f h", c=P, f=OUT_DIM),
                        vg4,
                        w2_sb[:, c0 * 8:(c0 + P) * 8].rearrange(
                            "p (c h) -> p c () h", c=P).to_broadcast([128, P, OUT_DIM, HEADS]),
                    )
                    for c in range(P):
                        nc.tensor.matmul(
                            num_ps[:],
                            lhsT=ident_sb[:],
                            rhs=wv_sb[:, c * 512:(c + 1) * 512],
                            start=(c0 + c == 0),
                            stop=(c0 + c == C - 1),
                        )

                out_sb = wk.tile([128, OUT_DIM], BF16, tag="out")
                if C > 0:
                    den_sb = wk.tile([128, 8], F32, tag="den")
                    nc.vector.reduce_sum(
                        den_sb[:],
                        w2_sb[:, :C * 8].rearrange("p (c h) -> p h c", c=C),
                        axis=mybir.AxisListType.X,
                    )
                    rec_sb = wk.tile([128, 8], BF16, tag="rec")
                    recf = wk.tile([128, 8], F32, tag="recf")
                    nc.vector.tensor_sub(
                        recf[:], den_sb[:],
                        sknp_sb[:, 64:65].to_broadcast([128, 8]),
                    )
                    nc.vector.tensor_scalar(
                        out=recf[:], in0=recf[:],
                        scalar1=float(HEADS), scalar2=1e-12,
                        op0=mybir.AluOpType.mult, op1=mybir.AluOpType.add,
                    )
                    with nc.allow_low_precision(reason="bf16 1/den keeps the mh multiply in DVE 2x mode"):
                        nc.vector.reciprocal(rec_sb[:], recf[:])
                    num_sb = wk.tile([128, HC], BF16, tag="numsb")
                    nc.scalar.copy(num_sb[:], num_ps[:])
                    mh_sb = wk.tile([128, HC], BF16, tag="mh")
                    nc.vector.tensor_mul(
                        mh_sb.rearrange("p (f h) -> p f h", f=OUT_DIM),
                        num_sb.rearrange("p (f h) -> p f h", f=OUT_DIM),
                        rec_sb.rearrange("p h -> p () h").to_broadcast([128, OUT_DIM, HEADS]),
                    )
                    msum_sb = wk.tile([128, OUT_DIM], F32, tag="msum")
                    nc.vector.reduce_sum(
                        msum_sb[:],
                        mh_sb.rearrange("p (f h) -> p f h", f=OUT_DIM),
                        axis=mybir.AxisListType.X,
                    )
                    nc.vector.tensor_add(out_sb[:], msum_sb[:], sknp_sb[:, :64])
                else:
                    nc.vector.tensor_copy(out_sb[:], sknp_sb[:, :64])
                nc.tensor.matmul(
                    pool_ps[:], lhsT=indng_sb[:], rhs=out_sb[:],
                    start=(t == 0), stop=(t == TILES - 1),
                )
            pooled_sb = outp.tile([B, OUT_DIM], F32)
            nc.vector.tensor_copy(pooled_sb[:], pool_ps[:])
            nc.sync.dma_start(pooled[:], pooled_sb[:])
    nc.compile()
    return nc


def _get_program_a():
    if "A" not in _cache:
        _cache["A"] = _build_launch_a()
    return _cache["A"]


def _get_program_b(c_list):
    key = ("B", tuple(c_list))
    if key not in _cache:
        _cache[key] = _build_launch_b(c_list)
    return _cache[key]


def _ensure_hook_shim():
    import sys
    import types

    if "antenv.axon_hooks" in sys.modules:
        return
    mod = types.ModuleType("antenv.axon_hooks")
    holder = [None]
    mod.set_axon_ntff_profile_hook = lambda h: holder.__setitem__(0, h)
    mod.get_axon_ntff_profile_hook = lambda: holder[0]
    sys.modules["antenv.axon_hooks"] = mod
    import antenv

    antenv.axon_hooks = mod
    from trn_agent_boot.trn_boot import _ntff_profile_via_ctypes

    mod.set_axon_ntff_profile_hook(
        _ntff_profile_via_ctypes("/opt/axon/libaxon_pjrt.so")
    )


def _run(nc, in_maps, label):
    if not TRACE:
        res = bass_utils.run_bass_kernel_spmd(nc, in_maps, list(range(NCORES)))
        return res.results

    import glob
    import os
    import tempfile

    from concourse import bass2jax
    from concourse._compat import FishPath
    import gauge.profiler

    _ensure_hook_shim()
    import antenv.axon_hooks as hooks

    tmpdir = tempfile.mkdtemp(prefix=f"bass_{label}_")
    with hooks.get_axon_ntff_profile_hook()(tmpdir, [0]):
        results = bass2jax.run_bass_via_pjrt(nc, in_maps, n_cores=NCORES)
    exec_ns = None
    try:
        ntffs = glob.glob(os.path.join(tmpdir, "*_body*.ntff"))
        if ntffs:
            profile = gauge.profiler.Profile(
                profile_path=FishPath(tmpdir),
                kernel_dev_mode=True,
                profile_on_exit=False,
                bass_kernel=nc.m,
                offline_processing=True,
                fname="*_body*",
            )
            prs = profile.to_perfetto(model_index=(0,))
            if prs:
                exec_ns = max(p.exec_time_ns for p in prs)
                LAST_TRACE_PATH[label] = (tmpdir, [p.trace_path for p in prs])
        else:
            print(f"[{label}] no ntff files in {tmpdir}: {os.listdir(tmpdir)}")
    except Exception as e:  # profiling must never break the run
        print(f"[{label}] profile processing failed: {type(e).__name__}: {e}")
    LAST_EXEC_NS[label] = exec_ns
    return results


def kernel(x, edge_index, batch, W_emb, b_emb, Wq, bq, Wk, bk, Wv, bv, Wskip, bskip):
    x = np.asarray(x, np.float32)
    edge_index = np.asarray(edge_index)
    batch_np = np.asarray(batch, np.int64)
    ncA = _get_program_a()

    # ---- host prep for launch A: fold W_emb/b_emb into the qkv/skip weights ----
    wcat = np.concatenate(
        [np.asarray(Wq, np.float32), np.asarray(Wk, np.float32),
         np.asarray(Wv, np.float32), np.asarray(Wskip, np.float32)], axis=1
    )  # [768, 1600]
    bcat = np.concatenate(
        [np.asarray(bq, np.float32), np.asarray(bk, np.float32),
         np.asarray(bv, np.float32), np.asarray(bskip, np.float32)]
    )  # [1600]
    wemb_f = np.asarray(W_emb, np.float32)
    bemb_f = np.asarray(b_emb, np.float32)
    wqkvs = (wemb_f @ wcat).astype(NP_BF16)          # [768, 1600]
    bqkvs = (bemb_f @ wcat + bcat).astype(np.float32)
    bqkvs_rep = np.broadcast_to(bqkvs.astype(NP_BF16), (128, 1600)).copy()

    xpad = np.zeros((NCORES * NPAD, IN_DIM), NP_BF16)
    for c in range(NCORES):
        xpad[c * NPAD: c * NPAD + NPC] = x[c * NPC:(c + 1) * NPC].astype(NP_BF16)
    in_maps_a = []
    for c in range(NCORES):
        xT = np.ascontiguousarray(xpad[c * NPAD:(c + 1) * NPAD].T)  # [768, 6272]
        in_maps_a.append({"xT": xT, "wqkvs": wqkvs, "bqkvs": bqkvs_rep})
    res_a = _run(ncA, in_maps_a, "A")

    # ---- host mid: assemble Q,K,V (fp8) + skip; build degree-grouped tiles ----
    qkv = np.concatenate([res_a[c]["qkv_out"][:NPC] for c in range(NCORES)])  # [N,1536] bf16
    SK = np.concatenate([res_a[c]["skip_out"][:NPC] for c in range(NCORES)])  # [N,64] f32

    Q8 = np.zeros((N + 1, HC), NP_F8)
    K8 = np.zeros((N + 1, HC), NP_F8)
    V8 = np.zeros((N + 1, HC), NP_F8)
    Q8[:N] = qkv[:, 0:512].astype(NP_F8)
    K8[:N] = qkv[:, 512:1024].astype(NP_F8)
    V8[:N] = qkv[:, 1024:1536].astype(NP_F8)

    src = np.asarray(edge_index[0], np.int64)
    dst = np.asarray(edge_index[1], np.int64)
    deg = np.bincount(dst, minlength=N)

    order = np.argsort(deg, kind="stable")  # ascending degree
    slot_node = np.full(NSLOT, N, np.int64)
    slot_node[176:] = order
    pos_of_node = np.empty(N, np.int64)
    pos_of_node[order] = 176 + np.arange(N)

    degslot = np.zeros(NSLOT, np.int64)
    degslot[176:] = deg[order]
    c_list = degslot.reshape(TILES, NCORES * 128).max(axis=1).tolist()
    c_list = [int(c) for c in c_list]
    choff = np.zeros(TILES + 1, np.int64)
    choff[1:] = np.cumsum(c_list)
    nch = int(choff[-1])

    # edge -> (core, tile, lane, chunk)
    pd = pos_of_node[dst]
    ecore = pd % NCORES
    er = pd // NCORES
    etile = er // 128
    elane = er % 128
    o = np.argsort(pd, kind="stable")
    pds = pd[o]
    uniq, grp_start = np.unique(pds, return_index=True)
    starts_per_edge = np.zeros(E, np.int64)
    starts_per_edge[grp_start] = grp_start
    starts_per_edge = np.maximum.accumulate(starts_per_edge)
    k_in_grp = np.arange(E) - starts_per_edge
    kchunk = np.empty(E, np.int64)
    kchunk[o] = k_in_grp

    src_grid = np.full((NCORES, nch, 128), N, np.int32)
    src_grid[ecore, choff[etile] + kchunk, elane] = src

    # node grid per (core, tile, lane)
    posg = (np.arange(TILES)[None, :, None] * (NCORES * 128)
            + np.arange(128)[None, None, :] * NCORES
            + np.arange(NCORES)[:, None, None])  # [core, tile, lane]
    node_grid = slot_node[posg]  # [NCORES, TILES, 128]

    SKz = np.zeros((N + 1, OUT_DIM), np.float32)
    SKz[:N] = SK
    degz = np.zeros(N + 1, np.int64)
    degz[:N] = deg
    batchz = np.full(N + 1, -1, np.int64)
    batchz[:N] = batch_np

    sknp_all = np.zeros((NCORES, TILES, 128, 66), np.float32)
    sknp_all[:, :, :, :64] = SKz[node_grid]
    sknp_all[:, :, :, 64] = (np.asarray(c_list)[None, :, None] - degz[node_grid])
    indng_all = (batchz[node_grid][:, :, :, None] == np.arange(B)[None, None, None, :]
                 ).astype(NP_BF16)

    hsel_np = (np.arange(128)[:, None] % 8 == np.arange(8)[None, :]).astype(NP_BF16)
    ident_np = np.eye(128, dtype=NP_BF16)
    i8_np = np.eye(8, dtype=NP_BF16)

    free_tot = sum(512 + c * 1024 for c in c_list)
    in_maps_b = []
    for c in range(NCORES):
        KG = K8[src_grid[c]]  # [nch, 128, 512] fp8
        VG = V8[src_grid[c]]
        kvq = np.empty((128, free_tot), NP_F8)
        fo = 0
        for t, C in enumerate(c_list):
            nodes_t = node_grid[c, t]  # [128]
            qrows = Q8[nodes_t]  # [128, 512]
            # qT: [d, h, b, l] -> [l, h, b, d] -> [128, 4*128]
            qT = qrows.reshape(128, 8, 4, 16).transpose(3, 1, 2, 0).reshape(128, 512)
            kvq[:, fo:fo + 512] = qT
            fo += 512
            for (p0, P) in _passes(C):
                kgp = KG[choff[t] + p0: choff[t] + p0 + P]  # [P, 128, 512]
                # kgT: [c, d, h, b, l] -> [l, h, b, c, d]
                kgT = kgp.reshape(P, 128, 8, 4, 16).transpose(4, 2, 3, 0, 1).reshape(
                    128, P * 512)
                kvq[:, fo:fo + P * 512] = kgT
                fo += P * 512
                vgp = VG[choff[t] + p0: choff[t] + p0 + P]  # [P, 128, 512]
                # vg c-major: [c, d, h, f] -> [d, c, f, h]
                vgc = vgp.reshape(P, 128, 8, 64).transpose(1, 0, 3, 2).reshape(
                    128, P * 512)
                kvq[:, fo:fo + P * 512] = vgc
                fo += P * 512
        assert fo == free_tot
        in_maps_b.append({
            "kvq": kvq,
            "sknp": sknp_all[c],
            "indng": indng_all[c],
            "hsel": hsel_np,
            "identt": ident_np,
            "i8t": i8_np,
        })

    ncB = _get_program_b(c_list)
    res_b = _run(ncB, in_maps_b, "B")

    pooled = np.zeros((B, OUT_DIM), np.float64)
    for c in range(NCORES):
        pooled += res_b[c]["pooled"].astype(np.float64)
    cnt = np.bincount(batch_np, minlength=B).astype(np.float64)
    pooled /= np.maximum(cnt, 1.0)[:, None]
    return pooled.astype(np.float32)


# revision 8
# speedup vs baseline: 1.5503x; 1.0767x over previous
"""GraphTransformer (TransformerConv + mean-pool) on 8 trn2 NeuronCores.

Strategy (two launches):
  Launch A (per core, 6250 nodes + pad -> 6272):
      qkv/skip = x @ fused(W_emb,[Wq|Wk|Wv|Wskip]) + fused bias   (bf16 TensorE)
  Host: assemble Q,K,V (cast fp8), sort nodes by dst-degree, stripe across
      8 cores into 49 tiles x 128 lanes; chunk c of a tile holds the c-th
      incident edge of each lane's node (pad rows zero).  Per-tile chunk
      count C_t = max degree in the tile's degree window (SPMD-identical
      across cores).  Ship per-edge K rows transposed (kgT, channel on
      partitions) and V rows channel-major (vg), plus Q transposed per tile.
  Launch B (per core, 49 tiles, passes of <=PCAP chunks):
      qk   = kgT * qT(bcast)                  DVE 2x
      s    = headsel^T @ qk                   TensorE (head-sum over channels)
      w    = exp(s*scale)                     ACT, [8, P*128] only
      w2   = transpose(w) via identity matmul TensorE -> [128, C*8]
      wv   = vg * w2(bcast)                   DVE 2x (c-major keeps stride-1)
      num += I128 @ wv                        TensorE PSUM accumulate
      den  = sum_c w2 - npad;  out = mean_h(num/(8*den)) + skip
      pooled += indng^T @ out                 TensorE per-graph partial
  Host: sum partial pooled over cores, divide by graph node counts.
"""

import math

import numpy as np
import ml_dtypes

import concourse.bass as bass
from concourse import bacc
import concourse.mybir as mybir
import concourse.tile as tile
from concourse import bass_utils
from concourse.bass import ts

BF16 = mybir.dt.bfloat16
F32 = mybir.dt.float32
F8 = mybir.dt.float8e4
NP_BF16 = ml_dtypes.bfloat16
NP_F8 = ml_dtypes.float8_e4m3fn

N, E, B = 50000, 400000, 64
IN_DIM, OUT_DIM, HEADS = 768, 64, 8
HC = HEADS * OUT_DIM  # 512
NCORES = 8
NPC = N // NCORES  # 6250 nodes per core (launch A sharding)
TILES = 49
NPAD = TILES * 128  # 6272
NSLOT = NCORES * NPAD  # 50176
KCH = IN_DIM // 128  # 6 contraction chunks (launch A)
PCAP = 12  # max chunks per pass in launch B

TRACE = False
LAST_EXEC_NS = {}
LAST_TRACE_PATH = {}

_cache = {}


WQK_SCALE = 32.0  # fp8 range lift for the fused q/k weights


def _build_launch_a():
    # qkv/skip computed directly from x with host-fused weights:
    #   W_eff = W_emb @ [Wq|Wk|Wv|Wskip],  b_eff = b_emb @ [..] + [bq|bk|bv|bskip]
    # q/k columns (1024) run as fp8 DoubleRow matmuls (weights pre-scaled by
    # WQK_SCALE, folded back in the bias epilogue); v/skip stay bf16.
    nc = bacc.Bacc("TRN2", debug=False, num_devices=NCORES)
    xT = nc.dram_tensor("xT", [KCH * 128, NPAD], BF16, kind="ExternalInput").ap()
    xT8 = nc.dram_tensor("xT8", [KCH * 128, NPAD], F8, kind="ExternalInput").ap()
    wqk8 = nc.dram_tensor("wqk8", [KCH * 128, 1024], F8, kind="ExternalInput").ap()
    wvs = nc.dram_tensor("wvs", [KCH * 128, 576], BF16, kind="ExternalInput").ap()
    bqkvs = nc.dram_tensor("bqkvs", [128, 1600], BF16, kind="ExternalInput").ap()
    qkv_out = nc.dram_tensor("qkv_out", [NPAD, 1536], BF16, kind="ExternalOutput").ap()
    skip_out = nc.dram_tensor("skip_out", [NPAD, OUT_DIM], F32, kind="ExternalOutput").ap()

    with tile.TileContext(nc) as tc:
        with (
            tc.tile_pool(name="const", bufs=1) as cpool,
            tc.tile_pool(name="work", bufs=3) as wpool,
            tc.tile_pool(name="psum_qkv", bufs=2, space="PSUM") as pq,
        ):
            xT_sb = cpool.tile([128, KCH * NPAD], BF16)
            xT8_sb = cpool.tile([128, KCH * NPAD], F8)
            wqk8_sb = cpool.tile([128, KCH * 1024], F8)
            wvs_sb = cpool.tile([128, KCH * 576], BF16)
            bqkvs_sb = cpool.tile([128, 1600], BF16)
            for k in range(KCH):
                nc.sync.dma_start(xT_sb[:, k * NPAD:(k + 1) * NPAD], xT[ts(k, 128), :])
                nc.sync.dma_start(xT8_sb[:, k * NPAD:(k + 1) * NPAD], xT8[ts(k, 128), :])
                nc.sync.dma_start(wqk8_sb[:, k * 1024:(k + 1) * 1024], wqk8[ts(k, 128), :])
                nc.sync.dma_start(wvs_sb[:, k * 576:(k + 1) * 576], wvs[ts(k, 128), :])
            nc.sync.dma_start(bqkvs_sb[:], bqkvs[:])
            xT8_v = xT8_sb.rearrange("p (k m) -> p k m", k=KCH)
            wqk8_v = wqk8_sb.rearrange("p (k n) -> p k n", k=KCH)

            for m in range(TILES):
                qkvs_ps = pq.tile([128, 1600], F32, tag="qkvs")
                for kk in range(KCH // 2):
                    for n0 in (0, 512):
                        nc.tensor.matmul(
                            qkvs_ps[:, n0:n0 + 512],
                            lhsT=xT8_v[:, 2 * kk:2 * kk + 2, ts(m, 128)],
                            rhs=wqk8_v[:, 2 * kk:2 * kk + 2, n0:n0 + 512],
                            start=(kk == 0),
                            stop=(kk == KCH // 2 - 1),
                            perf_mode=mybir.MatmulPerfMode.DoubleRow,
                        )
                for k in range(KCH):
                    for n0, nw in ((0, 512), (512, 64)):
                        nc.tensor.matmul(
                            qkvs_ps[:, 1024 + n0:1024 + n0 + nw],
                            lhsT=xT_sb[:, k * NPAD + m * 128: k * NPAD + (m + 1) * 128],
                            rhs=wvs_sb[:, k * 576 + n0: k * 576 + n0 + nw],
                            start=(k == 0),
                            stop=(k == KCH - 1),
                        )
                qkv_sb = wpool.tile([128, 1536], BF16, tag="qkv")
                nc.vector.scalar_tensor_tensor(
                    out=qkv_sb[:, :1024], in0=qkvs_ps[:, :1024],
                    scalar=1.0 / WQK_SCALE, in1=bqkvs_sb[:, :1024],
                    op0=mybir.AluOpType.mult, op1=mybir.AluOpType.add,
                )
                nc.vector.tensor_add(
                    qkv_sb[:, 1024:1536], qkvs_ps[:, 1024:1536], bqkvs_sb[:, 1024:1536])
                skip_sb = wpool.tile([128, OUT_DIM], F32, tag="skip")
                nc.vector.tensor_add(skip_sb[:], qkvs_ps[:, 1536:1600], bqkvs_sb[:, 1536:1600])
                nc.sync.dma_start(qkv_out[ts(m, 128), :], qkv_sb[:])
                nc.sync.dma_start(skip_out[ts(m, 128), :], skip_sb[:])
    nc.compile()
    return nc


def _passes(c):
    return [(p0, min(PCAP, c - p0)) for p0 in range(0, c, PCAP)]


def _build_launch_b(c_list):
    nc = bacc.Bacc("TRN2", debug=False, num_devices=NCORES)
    cmax = max(max(c_list), 1)
    free_tot = sum(512 + c * 1024 for c in c_list)
    kvq = nc.dram_tensor("kvq", [128, free_tot], F8, kind="ExternalInput").ap()
    sknp = nc.dram_tensor("sknp", [TILES, 128, 66], F32, kind="ExternalInput").ap()
    indng = nc.dram_tensor("indng", [TILES, 128, B], BF16, kind="ExternalInput").ap()
    hsel = nc.dram_tensor("hsel", [128, 8], BF16, kind="ExternalInput").ap()
    identt = nc.dram_tensor("identt", [128, 128], BF16, kind="ExternalInput").ap()
    i8t = nc.dram_tensor("i8t", [8, 8], BF16, kind="ExternalInput").ap()
    pooled = nc.dram_tensor("pooled", [B, OUT_DIM], F32, kind="ExternalOutput").ap()

    scale = 1.0 / math.sqrt(float(OUT_DIM))

    with tile.TileContext(nc) as tc:
        with (
            tc.tile_pool(name="const", bufs=1) as cp,
            tc.tile_pool(name="io", bufs=3) as iop,
            tc.tile_pool(name="tio", bufs=3) as tiop,
            tc.tile_pool(name="wk", bufs=3) as wk,
            tc.tile_pool(name="ps_s", bufs=3, space="PSUM") as ps_s,
            tc.tile_pool(name="ps_w2", bufs=2, space="PSUM") as ps_w2,
            tc.tile_pool(name="ps_num", bufs=2, space="PSUM") as ps_num,
            tc.tile_pool(name="ps_pool", bufs=1, space="PSUM") as ps_pool,
            tc.tile_pool(name="outp", bufs=1) as outp,
        ):
            hsel_sb = cp.tile([128, 8], BF16)
            nc.sync.dma_start(hsel_sb[:], hsel[:])
            ident_sb = cp.tile([128, 128], BF16)
            nc.sync.dma_start(ident_sb[:], identt[:])
            i8_sb = cp.tile([8, 8], BF16)
            nc.sync.dma_start(i8_sb[:], i8t[:])

            pool_ps = ps_pool.tile([B, OUT_DIM], F32)
            off = 0
            for t, C in enumerate(c_list):
                qt_sb = tiop.tile([128, 512], BF16, tag="qt")
                nc.gpsimd.dma_start(qt_sb[:], kvq[:, off:off + 512])
                off += 512
                sknp_sb = tiop.tile([128, 66], F32, tag="sknp")
                nc.sync.dma_start(sknp_sb[:], sknp[t])
                indng_sb = tiop.tile([128, B], BF16, tag="ing")
                nc.sync.dma_start(indng_sb[:], indng[t])

                w2_sb = wk.tile([128, cmax * 8], BF16, tag="w2")
                num_ps = ps_num.tile([128, HC], F32, tag="num")
                w2_ps = ps_w2.tile([128, cmax * 8], F32, tag="w2ps")
                qt4 = qt_sb.rearrange("p (b d) -> p b d", b=4)

                for (c0, P) in _passes(C):
                    kv_sb = iop.tile([128, PCAP * 1024], BF16, tag="kv")
                    nc.gpsimd.dma_start(kv_sb[:, :P * 1024], kvq[:, off:off + P * 1024])
                    off += P * 1024
                    kg4 = kv_sb[:, :P * 512].rearrange("p (b c d) -> p b c d", b=4, c=P)
                    vg4 = kv_sb[:, P * 512:P * 1024].rearrange(
                        "p (c f h) -> p c f h", c=P, f=OUT_DIM)

                    qk_sb = wk.tile([128, PCAP * 512], BF16, tag="qk")
                    qk4 = qk_sb[:, :P * 512].rearrange("p (b c d) -> p b c d", b=4, c=P)
                    nc.vector.tensor_mul(
                        qk4, kg4,
                        qt4.rearrange("p b d -> p b () d").to_broadcast([128, 4, P, 128]),
                    )
                    wT_sb = wk.tile([8, PCAP * 128], BF16, tag="wT")
                    for p0 in range(0, P, 4):
                        pw = min(4, P - p0)
                        s_ps = ps_s.tile([8, 512], F32, tag="s")
                        for bb in range(4):
                            nc.tensor.matmul(
                                s_ps[:, :pw * 128],
                                lhsT=hsel_sb[:],
                                rhs=qk4[:, bb, p0:p0 + pw],
                                start=(bb == 0),
                                stop=(bb == 3),
                            )
                        nc.scalar.activation(
                            out=wT_sb[:, p0 * 128:(p0 + pw) * 128],
                            in_=s_ps[:, :pw * 128],
                            func=mybir.ActivationFunctionType.Exp,
                            scale=float(scale),
                        )
                    for c in range(P):
                        nc.tensor.matmul(
                            w2_ps[:, (c0 + c) * 8:(c0 + c + 1) * 8],
                            lhsT=wT_sb[:, c * 128:(c + 1) * 128],
                            rhs=i8_sb[:],
                            start=True,
                            stop=True,
                        )
                    nc.scalar.copy(
                        w2_sb[:, c0 * 8:(c0 + P) * 8], w2_ps[:, c0 * 8:(c0 + P) * 8])
                    wv_sb = wk.tile([128, PCAP * 512], BF16, tag="wv")
                    nc.vector.tensor_mul(
                        wv_sb[:, :P * 512].rearrange("p (c f h) -> p c f h", c=P, f=OUT_DIM),
                        vg4,
                        w2_sb[:, c0 * 8:(c0 + P) * 8].rearrange(
                            "p (c h) -> p c () h", c=P).to_broadcast([128, P, OUT_DIM, HEADS]),
                    )
                    for c in range(P):
                        nc.tensor.matmul(
                            num_ps[:],
                            lhsT=ident_sb[:],
                            rhs=wv_sb[:, c * 512:(c + 1) * 512],
                            start=(c0 + c == 0),
                            stop=(c0 + c == C - 1),
                        )

                out_sb = wk.tile([128, OUT_DIM], BF16, tag="out")
                if C > 0:
                    den_sb = wk.tile([128, 8], F32, tag="den")
                    nc.vector.reduce_sum(
                        den_sb[:],
                        w2_sb[:, :C * 8].rearrange("p (c h) -> p h c", c=C),
                        axis=mybir.AxisListType.X,
                    )
                    rec_sb = wk.tile([128, 8], BF16, tag="rec")
                    recf = wk.tile([128, 8], F32, tag="recf")
                    nc.vector.tensor_sub(
                        recf[:], den_sb[:],
                        sknp_sb[:, 64:65].to_broadcast([128, 8]),
                    )
                    nc.vector.tensor_scalar(
                        out=recf[:], in0=recf[:],
                        scalar1=float(HEADS), scalar2=1e-12,
                        op0=mybir.AluOpType.mult, op1=mybir.AluOpType.add,
                    )
                    with nc.allow_low_precision(reason="bf16 1/den keeps the mh multiply in DVE 2x mode"):
                        nc.vector.reciprocal(rec_sb[:], recf[:])
                    num_sb = wk.tile([128, HC], BF16, tag="numsb")
                    nc.scalar.copy(num_sb[:], num_ps[:])
                    mh_sb = wk.tile([128, HC], BF16, tag="mh")
                    nc.vector.tensor_mul(
                        mh_sb.rearrange("p (f h) -> p f h", f=OUT_DIM),
                        num_sb.rearrange("p (f h) -> p f h", f=OUT_DIM),
                        rec_sb.rearrange("p h -> p () h").to_broadcast([128, OUT_DIM, HEADS]),
                    )
                    msum_sb = wk.tile([128, OUT_DIM], F32, tag="msum")
                    nc.vector.reduce_sum(
                        msum_sb[:],
                        mh_sb.rearrange("p (f h) -> p f h", f=OUT_DIM),
                        axis=mybir.AxisListType.X,
                    )
                    nc.vector.tensor_add(out_sb[:], msum_sb[:], sknp_sb[:, :64])
                else:
                    nc.vector.tensor_copy(out_sb[:], sknp_sb[:, :64])
                nc.tensor.matmul(
                    pool_ps[:], lhsT=indng_sb[:], rhs=out_sb[:],
                    start=(t == 0), stop=(t == TILES - 1),
                )
            pooled_sb = outp.tile([B, OUT_DIM], F32)
            nc.vector.tensor_copy(pooled_sb[:], pool_ps[:])
            nc.sync.dma_start(pooled[:], pooled_sb[:])
    nc.compile()
    return nc


def _get_program_a():
    if "A" not in _cache:
        _cache["A"] = _build_launch_a()
    return _cache["A"]


def _get_program_b(c_list):
    key = ("B", tuple(c_list))
    if key not in _cache:
        _cache[key] = _build_launch_b(c_list)
    return _cache[key]


def _ensure_hook_shim():
    import sys
    import types

    if "antenv.axon_hooks" in sys.modules:
        return
    mod = types.ModuleType("antenv.axon_hooks")
    holder = [None]
    mod.set_axon_ntff_profile_hook = lambda h: holder.__setitem__(0, h)
    mod.get_axon_ntff_profile_hook = lambda: holder[0]
    sys.modules["antenv.axon_hooks"] = mod
    import antenv

    antenv.axon_hooks = mod
    from trn_agent_boot.trn_boot import _ntff_profile_via_ctypes

    mod.set_axon_ntff_profile_hook(
        _ntff_profile_via_ctypes("/opt/axon/libaxon_pjrt.so")
    )


def _run(nc, in_maps, label):
    if not TRACE:
        res = bass_utils.run_bass_kernel_spmd(nc, in_maps, list(range(NCORES)))
        return res.results

    import glob
    import os
    import tempfile

    from concourse import bass2jax
    from concourse._compat import FishPath
    import gauge.profiler

    _ensure_hook_shim()
    import antenv.axon_hooks as hooks

    tmpdir = tempfile.mkdtemp(prefix=f"bass_{label}_")
    with hooks.get_axon_ntff_profile_hook()(tmpdir, [0]):
        results = bass2jax.run_bass_via_pjrt(nc, in_maps, n_cores=NCORES)
    exec_ns = None
    try:
        ntffs = glob.glob(os.path.join(tmpdir, "*_body*.ntff"))
        if ntffs:
            profile = gauge.profiler.Profile(
                profile_path=FishPath(tmpdir),
                kernel_dev_mode=True,
                profile_on_exit=False,
                bass_kernel=nc.m,
                offline_processing=True,
                fname="*_body*",
            )
            prs = profile.to_perfetto(model_index=(0,))
            if prs:
                exec_ns = max(p.exec_time_ns for p in prs)
                LAST_TRACE_PATH[label] = (tmpdir, [p.trace_path for p in prs])
        else:
            print(f"[{label}] no ntff files in {tmpdir}: {os.listdir(tmpdir)}")
    except Exception as e:  # profiling must never break the run
        print(f"[{label}] profile processing failed: {type(e).__name__}: {e}")
    LAST_EXEC_NS[label] = exec_ns
    return results


def kernel(x, edge_index, batch, W_emb, b_emb, Wq, bq, Wk, bk, Wv, bv, Wskip, bskip):
    x = np.asarray(x, np.float32)
    edge_index = np.asarray(edge_index)
    batch_np = np.asarray(batch, np.int64)
    ncA = _get_program_a()

    # ---- host prep for launch A: fold W_emb/b_emb into the qkv/skip weights ----
    wcat = np.concatenate(
        [np.asarray(Wq, np.float32), np.asarray(Wk, np.float32),
         np.asarray(Wv, np.float32), np.asarray(Wskip, np.float32)], axis=1
    )  # [768, 1600]
    bcat = np.concatenate(
        [np.asarray(bq, np.float32), np.asarray(bk, np.float32),
         np.asarray(bv, np.float32), np.asarray(bskip, np.float32)]
    )  # [1600]
    wemb_f = np.asarray(W_emb, np.float32)
    bemb_f = np.asarray(b_emb, np.float32)
    weff = wemb_f @ wcat                              # [768, 1600]
    wqk8 = (weff[:, :1024] * WQK_SCALE).astype(NP_F8)
    wvs = weff[:, 1024:].astype(NP_BF16)              # [768, 576]
    bqkvs = (bemb_f @ wcat + bcat).astype(np.float32)
    bqkvs_rep = np.broadcast_to(bqkvs.astype(NP_BF16), (128, 1600)).copy()

    xpad = np.zeros((NCORES * NPAD, IN_DIM), NP_BF16)
    for c in range(NCORES):
        xpad[c * NPAD: c * NPAD + NPC] = x[c * NPC:(c + 1) * NPC].astype(NP_BF16)
    in_maps_a = []
    for c in range(NCORES):
        xT = np.ascontiguousarray(xpad[c * NPAD:(c + 1) * NPAD].T)  # [768, 6272]
        in_maps_a.append({"xT": xT, "xT8": xT.astype(NP_F8),
                          "wqk8": wqk8, "wvs": wvs, "bqkvs": bqkvs_rep})
    res_a = _run(ncA, in_maps_a, "A")

    # ---- host mid: assemble Q,K,V (fp8) + skip; build degree-grouped tiles ----
    qkv = np.concatenate([res_a[c]["qkv_out"][:NPC] for c in range(NCORES)])  # [N,1536] bf16
    SK = np.concatenate([res_a[c]["skip_out"][:NPC] for c in range(NCORES)])  # [N,64] f32

    Q8 = np.zeros((N + 1, HC), NP_F8)
    K8 = np.zeros((N + 1, HC), NP_F8)
    V8 = np.zeros((N + 1, HC), NP_F8)
    Q8[:N] = qkv[:, 0:512].astype(NP_F8)
    K8[:N] = qkv[:, 512:1024].astype(NP_F8)
    V8[:N] = qkv[:, 1024:1536].astype(NP_F8)

    src = np.asarray(edge_index[0], np.int64)
    dst = np.asarray(edge_index[1], np.int64)
    deg = np.bincount(dst, minlength=N)

    order = np.argsort(deg, kind="stable")  # ascending degree
    slot_node = np.full(NSLOT, N, np.int64)
    slot_node[176:] = order
    pos_of_node = np.empty(N, np.int64)
    pos_of_node[order] = 176 + np.arange(N)

    degslot = np.zeros(NSLOT, np.int64)
    degslot[176:] = deg[order]
    c_list = degslot.reshape(TILES, NCORES * 128).max(axis=1).tolist()
    c_list = [int(c) for c in c_list]
    choff = np.zeros(TILES + 1, np.int64)
    choff[1:] = np.cumsum(c_list)
    nch = int(choff[-1])

    # edge -> (core, tile, lane, chunk)
    pd = pos_of_node[dst]
    ecore = pd % NCORES
    er = pd // NCORES
    etile = er // 128
    elane = er % 128
    o = np.argsort(pd, kind="stable")
    pds = pd[o]
    uniq, grp_start = np.unique(pds, return_index=True)
    starts_per_edge = np.zeros(E, np.int64)
    starts_per_edge[grp_start] = grp_start
    starts_per_edge = np.maximum.accumulate(starts_per_edge)
    k_in_grp = np.arange(E) - starts_per_edge
    kchunk = np.empty(E, np.int64)
    kchunk[o] = k_in_grp

    src_grid = np.full((NCORES, nch, 128), N, np.int32)
    src_grid[ecore, choff[etile] + kchunk, elane] = src

    # node grid per (core, tile, lane)
    posg = (np.arange(TILES)[None, :, None] * (NCORES * 128)
            + np.arange(128)[None, None, :] * NCORES
            + np.arange(NCORES)[:, None, None])  # [core, tile, lane]
    node_grid = slot_node[posg]  # [NCORES, TILES, 128]

    SKz = np.zeros((N + 1, OUT_DIM), np.float32)
    SKz[:N] = SK
    degz = np.zeros(N + 1, np.int64)
    degz[:N] = deg
    batchz = np.full(N + 1, -1, np.int64)
    batchz[:N] = batch_np

    sknp_all = np.zeros((NCORES, TILES, 128, 66), np.float32)
    sknp_all[:, :, :, :64] = SKz[node_grid]
    sknp_all[:, :, :, 64] = (np.asarray(c_list)[None, :, None] - degz[node_grid])
    indng_all = (batchz[node_grid][:, :, :, None] == np.arange(B)[None, None, None, :]
                 ).astype(NP_BF16)

    hsel_np = (np.arange(128)[:, None] % 8 == np.arange(8)[None, :]).astype(NP_BF16)
    ident_np = np.eye(128, dtype=NP_BF16)
    i8_np = np.eye(8, dtype=NP_BF16)

    free_tot = sum(512 + c * 1024 for c in c_list)
    in_maps_b = []
    for c in range(NCORES):
        KG = K8[src_grid[c]]  # [nch, 128, 512] fp8
        VG = V8[src_grid[c]]
        kvq = np.empty((128, free_tot), NP_F8)
        fo = 0
        for t, C in enumerate(c_list):
            nodes_t = node_grid[c, t]  # [128]
            qrows = Q8[nodes_t]  # [128, 512]
            # qT: [d, h, b, l] -> [l, h, b, d] -> [128, 4*128]
            qT = qrows.reshape(128, 8, 4, 16).transpose(3, 1, 2, 0).reshape(128, 512)
            kvq[:, fo:fo + 512] = qT
            fo += 512
            for (p0, P) in _passes(C):
                kgp = KG[choff[t] + p0: choff[t] + p0 + P]  # [P, 128, 512]
                # kgT: [c, d, h, b, l] -> [l, h, b, c, d]
                kgT = kgp.reshape(P, 128, 8, 4, 16).transpose(4, 2, 3, 0, 1).reshape(
                    128, P * 512)
                kvq[:, fo:fo + P * 512] = kgT
                fo += P * 512
                vgp = VG[choff[t] + p0: choff[t] + p0 + P]  # [P, 128, 512]
                # vg c-major: [c, d, h, f] -> [d, c, f, h]
                vgc = vgp.reshape(P, 128, 8, 64).transpose(1, 0, 3, 2).reshape(
                    128, P * 512)
                kvq[:, fo:fo + P * 512] = vgc
                fo += P * 512
        assert fo == free_tot
        in_maps_b.append({
            "kvq": kvq,
            "sknp": sknp_all[c],
            "indng": indng_all[c],
            "hsel": hsel_np,
            "identt": ident_np,
            "i8t": i8_np,
        })

    ncB = _get_program_b(c_list)
    res_b = _run(ncB, in_maps_b, "B")

    pooled = np.zeros((B, OUT_DIM), np.float64)
    for c in range(NCORES):
        pooled += res_b[c]["pooled"].astype(np.float64)
    cnt = np.bincount(batch_np, minlength=B).astype(np.float64)
    pooled /= np.maximum(cnt, 1.0)[:, None]
    return pooled.astype(np.float32)
